# revision 21
# baseline (speedup 1.0000x reference)
"""BayesianLinear (y = x @ (mu + softplus(rho) * eps).T + bias) on 8 TRN2 cores.

Column-parallel sharding: each core owns OUT_F/8 = 512 output features.

Host-side prep is pure layout/precision staging (no reference math):
  - x is cast to bf16 and pre-tiled into the SBUF layout the TensorEngine
    needs for its stationary operand: x_t[bt, pi, po, bi] = x[bt*128+bi,
    po*128+pi], so each 128-row batch tile is one contiguous 1 MiB DMA.
  - weight_mu/rho/eps shards are transposed to [in_f, o_sh], tiled per
    128-row K-block, and PACKED into one bf16-typed tensor per K-block
    (mu bf16 | eps bf16 | rho fp16-bits) so W^T construction costs a
    single 384 KiB DMA per K-block. mu/eps ship as bf16 (their info is
    rounded into the bf16 W anyway); rho ships as fp16 because softplus
    amplifies its quantization ~3x and fp16 keeps that negligible.

Device per core:
  1. For each K-block k (32): one packed param DMA (GPSIMD SWDGE queue),
     softplus(rho) = Ln(1 + Exp(rho)) on ACT (no Softplus LUT on TRN2;
     Exp and Ln share one table), mul/add on DVE writing bf16 straight
     into the resident W^T tile [128, 32, 512]. No transpose on device.
  2. bias row = bias_mu + softplus(bias_rho) * bias_eps, built mid-
     construction (so its latency chain doesn't head-of-line block any
     engine queue), then broadcast across partitions with one K=1 matmul
     against a ones row — placed AFTER the first matmul group in PE
     program order, since the in-order PE stream would otherwise stall
     on the bias chain.
  3. First 8 batch tiles run k-interleaved across all 8 PSUM banks so the
     PE consumes W^T blocks no faster than construction produces them;
     their x tiles are loaded chunk-major (first K-quarter of all strips
     first — Tile tracks deps at AP-range granularity). Remaining 56
     tiles stream one PSUM bank each: one 1 MiB x DMA, 32 accumulating
     bf16 matmuls into PSUM [128, 512] fp32, DVE eviction fused with the
     bias add, DMA out.
"""

import numpy as np
import ml_dtypes

import concourse.bacc as bacc
import concourse.mybir as mybir
import concourse.tile as tile
from concourse.bass_utils import run_bass_kernel_spmd

BATCH = 8192
IN_F = 4096
OUT_F = 4096
N_CORES = 8
P = 128

_NC_CACHE = {}


def build_nc(batch=BATCH, in_f=IN_F, o_sh=OUT_F // N_CORES):
    KB = in_f // P  # K-blocks of 128 along the contraction dim
    BT = batch // P  # 128-row output tiles

    nc = bacc.Bacc(
        "TRN2",
        target_bir_lowering=False,
        debug=False,
        enable_asserts=False,
        num_devices=N_CORES,
    )
    bf16 = mybir.dt.bfloat16
    f16 = mybir.dt.float16
    f32 = mybir.dt.float32

    x = nc.declare_dram_parameter("x_t", [BT, P, KB, P], bf16, isOutput=False)
    wpk = nc.declare_dram_parameter("wpk_t", [KB, P, 3 * o_sh], bf16, isOutput=False)
    bmu = nc.declare_dram_parameter("bias_mu", [1, o_sh], f32, isOutput=False)
    brho = nc.declare_dram_parameter("bias_rho", [1, o_sh], f32, isOutput=False)
    beps = nc.declare_dram_parameter("bias_eps", [1, o_sh], f32, isOutput=False)
    y = nc.declare_dram_parameter("y", [batch, o_sh], f32, isOutput=True)

    act_exp = mybir.ActivationFunctionType.Exp
    act_ln = mybir.ActivationFunctionType.Ln

    with tile.TileContext(nc) as tc:
        with (
            tc.tile_pool(name="const", bufs=1) as const,
            tc.tile_pool(name="wcons", bufs=5) as wcons,
            tc.tile_pool(name="xin", bufs=10) as xin,
            tc.tile_pool(name="yout", bufs=4) as yout,
            tc.tile_pool(name="psum", bufs=7, space="PSUM") as psum_pool,
            tc.tile_pool(name="bpsum", bufs=1, space="PSUM") as bias_psum,
        ):
            bias_sb = const.tile([P, o_sh], f32, tag="bias_sb")
            bias_bf = const.tile([1, o_sh], bf16, tag="bias_bf")
            ones = const.tile([1, P], bf16, tag="ones")
            nc.vector.memset(ones[:], 1.0)
            wones = const.tile([1, o_sh], bf16, tag="wones")
            nc.vector.memset(wones[:], 1.0)

            # PE warmup: dummy K=1 matmuls with no DMA deps keep the PE
            # HAM-busy through the first W^T block's latency chain, so the
            # real matmul stream starts at the warm 2.4 GHz clock instead
            # of paying ~12us of cold-clock inflation plus an idle gap.
            warm_ps = bias_psum.tile([P, o_sh], f32, tag="bias_ps", name="warm_ps")
            for w in range(28):
                nc.tensor.matmul(warm_ps[:], lhsT=ones[:], rhs=wones[:])

            def emit_bias_row():
                b_mu = const.tile([1, o_sh], f32, tag="b_mu")
                b_rho = const.tile([1, o_sh], f32, tag="b_rho")
                b_eps = const.tile([1, o_sh], f32, tag="b_eps")
                nc.gpsimd.dma_start(out=b_mu[:], in_=bmu[:])
                nc.gpsimd.dma_start(out=b_rho[:], in_=brho[:])
                nc.gpsimd.dma_start(out=b_eps[:], in_=beps[:])
                b_sp = const.tile([1, o_sh], f32, tag="b_sp")
                nc.scalar.activation(b_sp[:], b_rho[:], act_exp)
                nc.scalar.activation(b_sp[:], b_sp[:], act_ln, bias=1.0)
                nc.vector.tensor_mul(out=b_sp[:], in0=b_sp[:], in1=b_eps[:])
                nc.vector.tensor_add(out=bias_bf[:], in0=b_sp[:], in1=b_mu[:])

            # ---- W^T constructed in place, one packed DMA per K-block
            WT = const.tile([P, KB, o_sh], bf16, tag="WT")
            for k in range(KB):
                pk = wcons.tile([P, 3 * o_sh], bf16, tag="pk")
                nc.gpsimd.dma_start(out=pk[:], in_=wpk[k])
                mu_t = pk[:, 0:o_sh]
                eps_t = pk[:, o_sh : 2 * o_sh]
                rho_t = pk[:, 2 * o_sh : 3 * o_sh].bitcast(f16)
                sp_t = wcons.tile([P, o_sh], f32, tag="sp")
                nc.scalar.activation(sp_t[:], rho_t[:], act_exp)
                nc.scalar.activation(sp_t[:], sp_t[:], act_ln, bias=1.0)
                nc.vector.tensor_mul(out=sp_t[:], in0=sp_t[:], in1=eps_t[:])
                nc.vector.tensor_add(out=WT[:, k, :], in0=sp_t[:], in1=mu_t[:])
                if k == min(3, KB - 1):
                    emit_bias_row()

            def body_tail(ps, bt):
                y_sb = yout.tile([P, o_sh], f32, tag="y_sb")
                nc.vector.tensor_add(out=y_sb[:], in0=ps[:], in1=bias_sb[:])
                nc.sync.dma_start(out=y[bt * P : (bt + 1) * P, :], in_=y_sb[:])

            # ---- first GROUP tiles run k-interleaved across PSUM banks so
            # the PE consumes W^T blocks no faster than construction makes
            # them — the weight-construction latency hides under matmuls.
            GROUP = min(7, BT)
            xts = []
            pss = []
            for bt in range(GROUP):
                xT = xin.tile([P, KB, P], bf16, tag="xT", name=f"xT_g{bt}")
                xts.append(xT)
                ps = psum_pool.tile([P, o_sh], f32, tag="ps", name=f"ps_g{bt}")
                pss.append(ps)
            # chunk-major strip loads: the first K-quarter of every strip
            # lands before any second quarter, so the k=0 matmul batch isn't
            # gated on the last strip's full 1 MiB transfer.
            CH = 4 if KB % 4 == 0 else 1
            for c in range(CH):
                ks = slice(c * (KB // CH), (c + 1) * (KB // CH))
                for i in range(GROUP):
                    nc.sync.dma_start(out=xts[i][:, ks, :], in_=x[i, :, ks, :])
            for k in range(KB):
                for i in range(GROUP):
                    nc.tensor.matmul(
                        pss[i][:],
                        lhsT=xts[i][:, k, :],
                        rhs=WT[:, k, :],
                        start=(k == 0),
                        stop=(k == KB - 1),
                    )
                if k == min(8, KB - 1):
                    # bias broadcast: [128, o_sh] = ones.T @ bias_bf. Mid-
                    # stream (bias_bf is ready by now) so bias_sb exists
                    # before the first group eviction — the in-order PE
                    # stream must not head-of-line block on the bias chain.
                    bias_ps = bias_psum.tile(
                        [P, o_sh], f32, tag="bias_ps", name="bias_ps"
                    )
                    nc.tensor.matmul(bias_ps[:], lhsT=ones[:], rhs=bias_bf[:])
                    nc.vector.tensor_copy(out=bias_sb[:], in_=bias_ps[:])

            for i in range(GROUP):
                body_tail(pss[i], i)

            # ---- remaining tiles stream one PSUM bank each
            for bt in range(GROUP, BT):
                xT = xin.tile([P, KB, P], bf16, tag="xT")
                nc.sync.dma_start(out=xT[:], in_=x[bt])
                ps = psum_pool.tile([P, o_sh], f32, tag="ps")
                for k in range(KB):
                    nc.tensor.matmul(
                        ps[:],
                        lhsT=xT[:, k, :],
                        rhs=WT[:, k, :],
                        start=(k == 0),
                        stop=(k == KB - 1),
                    )
                body_tail(ps, bt)

    # Skip bacc's pre-placed InstLoadActFuncSet: on large graphs walrus's
    # parallel-pass fork can separate the hoisted load from its activations
    # ("No Act func set exist for this instruction"); walrus's own lower_act
    # placement handles forked subgraphs correctly.
    nc.insert_act_table_loads = lambda: None
    nc.compile()
    return nc


def _prep_x(x):
    """[batch, in_f] fp32 -> bf16 tiled [BT, 128, KB, 128] with
    x_t[bt, pi, po, bi] = x[bt*128 + bi, po*128 + pi]."""
    batch, in_f = x.shape
    xb = x.astype(ml_dtypes.bfloat16)
    xb = xb.reshape(batch // P, P, in_f // P, P)  # [bt, bi, po, pi]
    return np.ascontiguousarray(xb.transpose(0, 3, 2, 1))  # [bt, pi, po, bi]


def _tile_w(w, dtype):
    """[o_sh, in_f] -> tiled [KB, 128, o_sh] with w_t[k, pi, o] = w[o, k*128 + pi]."""
    o_sh, in_f = w.shape
    return np.ascontiguousarray(w.T.reshape(in_f // P, P, o_sh)).astype(dtype)


def _prep_wpk(wmu, wrho, weps):
    """Pack mu (bf16), eps (bf16), rho (fp16 bits viewed as bf16) into one
    bf16-typed [KB, 128, 3*o_sh] tensor — a single DMA per K-block."""
    mu = _tile_w(wmu, ml_dtypes.bfloat16)
    eps = _tile_w(weps, ml_dtypes.bfloat16)
    rho = _tile_w(wrho, np.float16).view(ml_dtypes.bfloat16)
    return np.ascontiguousarray(np.concatenate([mu, eps, rho], axis=2))


def make_in_maps(x, weight_mu, weight_rho, bias_mu, bias_rho, weight_eps, bias_eps):
    o_sh = OUT_F // N_CORES
    x_t = _prep_x(np.asarray(x, dtype=np.float32))
    wmu = np.asarray(weight_mu, dtype=np.float32)
    wrho = np.asarray(weight_rho, dtype=np.float32)
    weps = np.asarray(weight_eps, dtype=np.float32)
    bmu = np.asarray(bias_mu, dtype=np.float32).reshape(1, -1)
    brho = np.asarray(bias_rho, dtype=np.float32).reshape(1, -1)
    beps = np.asarray(bias_eps, dtype=np.float32).reshape(1, -1)

    in_maps = []
    for c in range(N_CORES):
        rs = slice(c * o_sh, (c + 1) * o_sh)
        in_maps.append(
            {
                "x_t": x_t,
                "wpk_t": _prep_wpk(wmu[rs], wrho[rs], weps[rs]),
                "bias_mu": np.ascontiguousarray(bmu[:, rs]),
                "bias_rho": np.ascontiguousarray(brho[:, rs]),
                "bias_eps": np.ascontiguousarray(beps[:, rs]),
            }
        )
    return in_maps


def kernel(x, weight_mu, weight_rho, bias_mu, bias_rho, weight_eps, bias_eps):
    o_sh = OUT_F // N_CORES
    key = (x.shape, o_sh)
    if key not in _NC_CACHE:
        _NC_CACHE[key] = build_nc(x.shape[0], x.shape[1], o_sh)
    nc = _NC_CACHE[key]

    in_maps = make_in_maps(
        x, weight_mu, weight_rho, bias_mu, bias_rho, weight_eps, bias_eps
    )
    res = run_bass_kernel_spmd(nc, in_maps, core_ids=list(range(N_CORES)))
    return np.concatenate([res.results[c]["y"] for c in range(N_CORES)], axis=1)


# revision 22
# speedup vs baseline: 1.0045x; 1.0045x over previous
"""BayesianLinear (y = x @ (mu + softplus(rho) * eps).T + bias) on 8 TRN2 cores.

Column-parallel sharding: each core owns OUT_F/8 = 512 output features.

Host-side prep is pure layout/precision staging (no reference math):
  - x is cast to bf16 and pre-tiled into the SBUF layout the TensorEngine
    needs for its stationary operand: x_t[bt, pi, po, bi] = x[bt*128+bi,
    po*128+pi], so each 128-row batch tile is one contiguous 1 MiB DMA.
  - weight_mu/rho/eps shards are transposed to [in_f, o_sh], tiled per
    128-row K-block, and PACKED into one bf16-typed tensor per K-block
    (mu bf16 | eps bf16 | rho fp16-bits) so W^T construction costs a
    single 384 KiB DMA per K-block. mu/eps ship as bf16 (their info is
    rounded into the bf16 W anyway); rho ships as fp16 because softplus
    amplifies its quantization ~3x and fp16 keeps that negligible.

Device per core:
  1. For each K-block k (32): one packed param DMA (GPSIMD SWDGE queue),
     softplus(rho) = Ln(1 + Exp(rho)) on ACT (no Softplus LUT on TRN2;
     Exp and Ln share one table), mul/add on DVE writing bf16 straight
     into the resident W^T tile [128, 32, 512]. No transpose on device.
  2. bias row = bias_mu + softplus(bias_rho) * bias_eps, built mid-
     construction (so its latency chain doesn't head-of-line block any
     engine queue), then broadcast across partitions with one K=1 matmul
     against a ones row — placed AFTER the first matmul group in PE
     program order, since the in-order PE stream would otherwise stall
     on the bias chain.
  3. First 8 batch tiles run k-interleaved across all 8 PSUM banks so the
     PE consumes W^T blocks no faster than construction produces them;
     their x tiles are loaded chunk-major (first K-quarter of all strips
     first — Tile tracks deps at AP-range granularity). Remaining 56
     tiles stream one PSUM bank each: one 1 MiB x DMA, 32 accumulating
     bf16 matmuls into PSUM [128, 512] fp32, DVE eviction fused with the
     bias add, DMA out.
"""

import numpy as np
import ml_dtypes

import concourse.bacc as bacc
import concourse.mybir as mybir
import concourse.tile as tile
from concourse.bass_utils import run_bass_kernel_spmd

BATCH = 8192
IN_F = 4096
OUT_F = 4096
N_CORES = 8
P = 128

_NC_CACHE = {}


def build_nc(batch=BATCH, in_f=IN_F, o_sh=OUT_F // N_CORES):
    KB = in_f // P  # K-blocks of 128 along the contraction dim
    BT = batch // P  # 128-row output tiles

    nc = bacc.Bacc(
        "TRN2",
        target_bir_lowering=False,
        debug=False,
        enable_asserts=False,
        num_devices=N_CORES,
    )
    bf16 = mybir.dt.bfloat16
    f16 = mybir.dt.float16
    f32 = mybir.dt.float32

    x = nc.declare_dram_parameter("x_t", [BT, P, KB, P], bf16, isOutput=False)
    K2 = 2 if KB % 2 == 0 else 1  # K-blocks per construction step
    wpk = nc.declare_dram_parameter(
        "wpk_t", [KB // K2, P, K2, 3 * o_sh], bf16, isOutput=False
    )
    bmu = nc.declare_dram_parameter("bias_mu", [1, o_sh], f32, isOutput=False)
    brho = nc.declare_dram_parameter("bias_rho", [1, o_sh], f32, isOutput=False)
    beps = nc.declare_dram_parameter("bias_eps", [1, o_sh], f32, isOutput=False)
    y = nc.declare_dram_parameter("y", [batch, o_sh], f32, isOutput=True)

    act_exp = mybir.ActivationFunctionType.Exp
    act_ln = mybir.ActivationFunctionType.Ln

    with tile.TileContext(nc) as tc:
        with (
            tc.tile_pool(name="const", bufs=1) as const,
            tc.tile_pool(name="wcons", bufs=4) as wcons,
            tc.tile_pool(name="xin", bufs=10) as xin,
            tc.tile_pool(name="yout", bufs=4) as yout,
            tc.tile_pool(name="psum", bufs=7, space="PSUM") as psum_pool,
            tc.tile_pool(name="bpsum", bufs=1, space="PSUM") as bias_psum,
        ):
            bias_sb = const.tile([P, o_sh], f32, tag="bias_sb")
            bias_bf = const.tile([1, o_sh], bf16, tag="bias_bf")
            ones = const.tile([1, P], bf16, tag="ones")
            nc.vector.memset(ones[:], 1.0)
            wones = const.tile([1, o_sh], bf16, tag="wones")
            nc.vector.memset(wones[:], 1.0)

            # PE warmup: dummy K=1 matmuls with no DMA deps keep the PE
            # HAM-busy through the first W^T block's latency chain, so the
            # real matmul stream starts at the warm 2.4 GHz clock instead
            # of paying ~12us of cold-clock inflation plus an idle gap.
            warm_ps = bias_psum.tile([P, o_sh], f32, tag="bias_ps", name="warm_ps")
            for w in range(28):
                nc.tensor.matmul(warm_ps[:], lhsT=ones[:], rhs=wones[:])

            def emit_bias_row():
                b_mu = const.tile([1, o_sh], f32, tag="b_mu")
                b_rho = const.tile([1, o_sh], f32, tag="b_rho")
                b_eps = const.tile([1, o_sh], f32, tag="b_eps")
                nc.gpsimd.dma_start(out=b_mu[:], in_=bmu[:])
                nc.gpsimd.dma_start(out=b_rho[:], in_=brho[:])
                nc.gpsimd.dma_start(out=b_eps[:], in_=beps[:])
                b_sp = const.tile([1, o_sh], f32, tag="b_sp")
                nc.scalar.activation(b_sp[:], b_rho[:], act_exp)
                nc.scalar.activation(b_sp[:], b_sp[:], act_ln, bias=1.0)
                nc.vector.tensor_mul(out=b_sp[:], in0=b_sp[:], in1=b_eps[:])
                nc.vector.tensor_add(out=bias_bf[:], in0=b_sp[:], in1=b_mu[:])

            # ---- W^T constructed in place, one packed DMA per K2 blocks
            # (pairing K-blocks halves the per-op ACT/DVE fixed overhead and
            # the DMA trigger count, so production outruns the PE's warm
            # consumption during the overlap group).
            WT = const.tile([P, KB, o_sh], bf16, tag="WT")
            for k2 in range(KB // K2):
                pk = wcons.tile([P, K2, 3 * o_sh], bf16, tag="pk")
                nc.gpsimd.dma_start(out=pk[:], in_=wpk[k2])
                mu_t = pk[:, :, 0:o_sh]
                eps_t = pk[:, :, o_sh : 2 * o_sh]
                rho_t = pk[:, :, 2 * o_sh : 3 * o_sh].bitcast(f16)
                sp_t = wcons.tile([P, K2, o_sh], f32, tag="sp")
                nc.scalar.activation(sp_t[:], rho_t[:], act_exp)
                nc.scalar.activation(sp_t[:], sp_t[:], act_ln, bias=1.0)
                nc.vector.tensor_mul(out=sp_t[:], in0=sp_t[:], in1=eps_t[:])
                nc.vector.tensor_add(
                    out=WT[:, k2 * K2 : (k2 + 1) * K2, :], in0=sp_t[:], in1=mu_t[:]
                )
                if k2 == min(1, KB // K2 - 1):
                    emit_bias_row()

            def body_tail(ps, bt):
                y_sb = yout.tile([P, o_sh], f32, tag="y_sb")
                nc.vector.tensor_add(out=y_sb[:], in0=ps[:], in1=bias_sb[:])
                nc.sync.dma_start(out=y[bt * P : (bt + 1) * P, :], in_=y_sb[:])

            # ---- first GROUP tiles run k-interleaved across PSUM banks so
            # the PE consumes W^T blocks no faster than construction makes
            # them — the weight-construction latency hides under matmuls.
            GROUP = min(7, BT)
            xts = []
            pss = []
            for bt in range(GROUP):
                xT = xin.tile([P, KB, P], bf16, tag="xT", name=f"xT_g{bt}")
                xts.append(xT)
                ps = psum_pool.tile([P, o_sh], f32, tag="ps", name=f"ps_g{bt}")
                pss.append(ps)
            # chunk-major strip loads: the first K-quarter of every strip
            # lands before any second quarter, so the k=0 matmul batch isn't
            # gated on the last strip's full 1 MiB transfer.
            CH = 4 if KB % 4 == 0 else 1
            for c in range(CH):
                ks = slice(c * (KB // CH), (c + 1) * (KB // CH))
                for i in range(GROUP):
                    nc.sync.dma_start(out=xts[i][:, ks, :], in_=x[i, :, ks, :])
            for k in range(KB):
                for i in range(GROUP):
                    nc.tensor.matmul(
                        pss[i][:],
                        lhsT=xts[i][:, k, :],
                        rhs=WT[:, k, :],
                        start=(k == 0),
                        stop=(k == KB - 1),
                    )
                if k == min(8, KB - 1):
                    # bias broadcast: [128, o_sh] = ones.T @ bias_bf. Mid-
                    # stream (bias_bf is ready by now) so bias_sb exists
                    # before the first group eviction — the in-order PE
                    # stream must not head-of-line block on the bias chain.
                    bias_ps = bias_psum.tile(
                        [P, o_sh], f32, tag="bias_ps", name="bias_ps"
                    )
                    nc.tensor.matmul(bias_ps[:], lhsT=ones[:], rhs=bias_bf[:])
                    nc.vector.tensor_copy(out=bias_sb[:], in_=bias_ps[:])

            for i in range(GROUP):
                body_tail(pss[i], i)

            # ---- remaining tiles stream one PSUM bank each
            for bt in range(GROUP, BT):
                xT = xin.tile([P, KB, P], bf16, tag="xT")
                nc.sync.dma_start(out=xT[:], in_=x[bt])
                ps = psum_pool.tile([P, o_sh], f32, tag="ps")
                for k in range(KB):
                    nc.tensor.matmul(
                        ps[:],
                        lhsT=xT[:, k, :],
                        rhs=WT[:, k, :],
                        start=(k == 0),
                        stop=(k == KB - 1),
                    )
                body_tail(ps, bt)

    # Skip bacc's pre-placed InstLoadActFuncSet: on large graphs walrus's
    # parallel-pass fork can separate the hoisted load from its activations
    # ("No Act func set exist for this instruction"); walrus's own lower_act
    # placement handles forked subgraphs correctly.
    nc.insert_act_table_loads = lambda: None
    nc.compile()
    return nc


def _prep_x(x):
    """[batch, in_f] fp32 -> bf16 tiled [BT, 128, KB, 128] with
    x_t[bt, pi, po, bi] = x[bt*128 + bi, po*128 + pi]."""
    batch, in_f = x.shape
    xb = x.astype(ml_dtypes.bfloat16)
    xb = xb.reshape(batch // P, P, in_f // P, P)  # [bt, bi, po, pi]
    return np.ascontiguousarray(xb.transpose(0, 3, 2, 1))  # [bt, pi, po, bi]


def _tile_w(w, dtype):
    """[o_sh, in_f] -> tiled [KB, 128, o_sh] with w_t[k, pi, o] = w[o, k*128 + pi]."""
    o_sh, in_f = w.shape
    return np.ascontiguousarray(w.T.reshape(in_f // P, P, o_sh)).astype(dtype)


def _prep_wpk(wmu, wrho, weps):
    """Pack mu (bf16), eps (bf16), rho (fp16 bits viewed as bf16) into one
    bf16-typed [KB/K2, 128, K2, 3*o_sh] tensor — one DMA per K2 K-blocks."""
    mu = _tile_w(wmu, ml_dtypes.bfloat16)
    eps = _tile_w(weps, ml_dtypes.bfloat16)
    rho = _tile_w(wrho, np.float16).view(ml_dtypes.bfloat16)
    pk = np.concatenate([mu, eps, rho], axis=2)  # [KB, P, 3*o_sh]
    kb, p, f = pk.shape
    k2 = 2 if kb % 2 == 0 else 1
    pk = pk.reshape(kb // k2, k2, p, f).transpose(0, 2, 1, 3)
    return np.ascontiguousarray(pk)


def make_in_maps(x, weight_mu, weight_rho, bias_mu, bias_rho, weight_eps, bias_eps):
    o_sh = OUT_F // N_CORES
    x_t = _prep_x(np.asarray(x, dtype=np.float32))
    wmu = np.asarray(weight_mu, dtype=np.float32)
    wrho = np.asarray(weight_rho, dtype=np.float32)
    weps = np.asarray(weight_eps, dtype=np.float32)
    bmu = np.asarray(bias_mu, dtype=np.float32).reshape(1, -1)
    brho = np.asarray(bias_rho, dtype=np.float32).reshape(1, -1)
    beps = np.asarray(bias_eps, dtype=np.float32).reshape(1, -1)

    in_maps = []
    for c in range(N_CORES):
        rs = slice(c * o_sh, (c + 1) * o_sh)
        in_maps.append(
            {
                "x_t": x_t,
                "wpk_t": _prep_wpk(wmu[rs], wrho[rs], weps[rs]),
                "bias_mu": np.ascontiguousarray(bmu[:, rs]),
                "bias_rho": np.ascontiguousarray(brho[:, rs]),
                "bias_eps": np.ascontiguousarray(beps[:, rs]),
            }
        )
    return in_maps


def kernel(x, weight_mu, weight_rho, bias_mu, bias_rho, weight_eps, bias_eps):
    o_sh = OUT_F // N_CORES
    key = (x.shape, o_sh)
    if key not in _NC_CACHE:
        _NC_CACHE[key] = build_nc(x.shape[0], x.shape[1], o_sh)
    nc = _NC_CACHE[key]

    in_maps = make_in_maps(
        x, weight_mu, weight_rho, bias_mu, bias_rho, weight_eps, bias_eps
    )
    res = run_bass_kernel_spmd(nc, in_maps, core_ids=list(range(N_CORES)))
    return np.concatenate([res.results[c]["y"] for c in range(N_CORES)], axis=1)
